# revision 4
# baseline (speedup 1.0000x reference)
"""DistSageConv forward on 8 Trainium2 NeuronCores (Bass/Tile).

Math per graph partition p (of 4):
    ng  = segment_sum(x[edge_src], edge_dst, NDST)          # neighbor agg
    h   = concat([x[self_ids], ng], axis=1)
    out = h[owned_ids] @ W.T + b
      == x[self_ids[owned_ids]] @ W1.T + ng[owned_ids] @ W2.T + b
         (W1 = W[:, :DIN], W2 = W[:, DIN:])

Only dst nodes that appear in owned_ids matter, so edges whose dst is not
owned are dropped on the host (~60% of them). Each partition is split
across 2 cores by interleaving its unique owned dst ids ("segments").

Per core the device program is:
  phase 1, per block of 128 segments:
    - dma_gather the block's edge source rows from x (4 SWDGE queues,
      edges grouped by src chunk of 25k rows to fit int16 indices)
    - build a one-hot selection matrix SelT[e, s] = (seg_local[e] == s)
      with a vector is_equal against a host-supplied iota tile
    - PE matmul accumulate ngT[din, seg] += xs_tile.T @ SelT  (PSUM)
    - indirect-DMA gather the block's 128 self rows (partition = seg)
    - selfT = PE transpose(xself);  zT = W2T.T @ ngT + W1T.T @ selfT
    - z = PE transpose(zT + b)  ->  DMA to z_dram[block*128 : ...]
  phase 2:
    - dma_gather z rows for this core's output rows, write them to the
      output buffer in a host-chosen order (host unpermutes at the end).
"""
import os
import numpy as np

import concourse.bass as bass
import concourse.bacc as bacc
import concourse.mybir as mybir
from concourse.tile import TileContext

F32 = mybir.dt.float32
I32 = mybir.dt.int32
I16 = mybir.dt.int16

# Tile's sem assignment round-robins SWDGE DMA insts across DMASW lanes
# with no regard for queue_num, but each DMA semaphore may only be updated
# from one SWDGE queue. Pin lane = queue_num so multi-queue gathers are
# legal. (Insts without queue_num, e.g. indirect_dma_start on qPoolDynamic,
# run on SWDGE queue 0 and get lane 0.)
import concourse.tile_sem_assignment as _tsa

if not getattr(_tsa, "_queue_lane_patch", False):
    _orig_assign_tick = _tsa.TileClockTick._assign_tick

    def _assign_tick_queue_aware(self, inst):
        if (
            isinstance(inst, _tsa.DMAInst)
            and inst.engine == mybir.EngineType.Pool
        ):
            self.next_sw_dma_idx = getattr(inst, "queue_num", 0) or 0
        return _orig_assign_tick(self, inst)

    _tsa.TileClockTick._assign_tick = _assign_tick_queue_aware
    _tsa._queue_lane_patch = True

NCORES = 8
LAST_EXEC_NS = None
SEG_BLK = 128          # segments per block (= psum tile free dim)
CHUNK = 25000          # src rows per dma_gather chunk (int16 idx < 32768)
OUT_GATHER = 1024      # rows per phase-2 dma_gather


def _wrap16(flat):
    """dma_gather index layout: idx i -> [i % 16, i // 16], replicated to
    all 8 groups of 16 partitions. flat length must be a multiple of 16."""
    n = len(flat)
    w = flat.reshape(n // 16, 16).T          # [16, n/16]
    return np.tile(w, (8, 1))                # [128, n/16]


def _prep_core(es, ed, sid, oid, half, ndst):
    """Host-side index prep for one core (partition p, half h)."""
    uniq = np.unique(oid)
    U = uniq[half::2]                         # this core's segments, sorted
    nu = len(U)
    seg_of_dst = np.full(ndst, -1, np.int32)
    seg_of_dst[U] = np.arange(nu, dtype=np.int32)

    seg_all = seg_of_dst[ed]
    keep = seg_all >= 0
    es_k = es[keep].astype(np.int64)
    seg_k = seg_all[keep].astype(np.int64)
    blk = seg_k // SEG_BLK
    loc = (seg_k % SEG_BLK).astype(np.float32)
    ch = es_k // CHUNK

    order = np.lexsort((ch, blk))
    es_o = (es_k - ch * CHUNK)[order]
    loc_o = loc[order]
    key_o = (blk * 4 + ch)[order]             # sorted ascending

    self_src = sid[U]                          # [nu]

    seg_out = seg_of_dst[oid]                  # [NOWN], -1 if other core's
    mine = seg_out >= 0
    rows = np.nonzero(mine)[0]                 # original output positions
    oseg = seg_out[mine].astype(np.int64)      # global seg id per out row

    return dict(nu=nu, es=es_o, loc=loc_o, key=key_o,
                self_src=self_src, rows=rows, oseg=oseg)


def _build_streams(prep, nb, gsz, t2):
    """Per-core device input arrays for the fixed (nb, gsz, t2) structure."""
    nb4 = nb * 4
    # grouped edge arrays [nb4, gsz]
    G = np.zeros((nb4, gsz), np.int16)
    S = np.full((nb4, gsz), -1.0, np.float32)
    key, es, loc = prep["key"], prep["es"], prep["loc"]
    starts = np.searchsorted(key, np.arange(nb4 + 1))
    ofs = np.arange(len(key)) - starts[key]
    G[key, ofs] = es.astype(np.int16)
    S[key, ofs] = loc

    # gather idx stream: per (b,ch) slab of gsz/16 cols, wrapped
    gidx = np.concatenate([_wrap16(G[i]) for i in range(nb4)], axis=1)
    # seg stream: [128, nb4 * gsz/128], col = slab*(gsz/128) + tile
    segs = np.ascontiguousarray(
        S.reshape(nb4, gsz // SEG_BLK, SEG_BLK).transpose(2, 0, 1).reshape(128, -1)
    )

    # self stream [128, nb] int32 (partition = seg local, col = block)
    selfidx = np.zeros((nb * SEG_BLK,), np.int32)
    selfidx[: prep["nu"]] = prep["self_src"]
    selfidx = np.ascontiguousarray(selfidx.reshape(nb, SEG_BLK).T)

    # phase-2 gather idx [128, t2*128/16] int16 (z rows = global seg id)
    oflat = np.zeros((t2 * SEG_BLK,), np.int64)
    oflat[: len(prep["oseg"])] = prep["oseg"]
    ngath = (t2 * SEG_BLK) // OUT_GATHER
    oidx = np.concatenate(
        [_wrap16(oflat[g * OUT_GATHER : (g + 1) * OUT_GATHER].astype(np.int16))
         for g in range(ngath)],
        axis=1,
    )
    return dict(gidx=np.ascontiguousarray(gidx), segs=segs,
                selfidx=selfidx, oidx=np.ascontiguousarray(oidx))


def _build_program(nsrc, din, dout, nb, gsz, t2):
    nc = bacc.Bacc(num_swdge_queues=4)
    tg = gsz // 128                      # tiles per (block, chunk) gather
    nb4cols = nb * 4 * (gsz // 16)
    x_d = nc.dram_tensor("x", [nsrc, din], F32, kind="ExternalInput")
    gidx_d = nc.dram_tensor("gidx", [128, nb4cols], I16, kind="ExternalInput")
    segs_d = nc.dram_tensor("segs", [128, nb * 4 * tg], F32, kind="ExternalInput")
    selfidx_d = nc.dram_tensor("selfidx", [128, nb], I32, kind="ExternalInput")
    ngath = (t2 * SEG_BLK) // OUT_GATHER
    oidx_d = nc.dram_tensor("oidx", [128, ngath * (OUT_GATHER // 16)], I16,
                            kind="ExternalInput")
    w1t_d = nc.dram_tensor("w1t", [din, dout], F32, kind="ExternalInput")
    w2t_d = nc.dram_tensor("w2t", [din, dout], F32, kind="ExternalInput")
    bias_d = nc.dram_tensor("bias", [dout, 1], F32, kind="ExternalInput")
    iota_d = nc.dram_tensor("iota", [128, SEG_BLK], F32, kind="ExternalInput")
    eye_d = nc.dram_tensor("eye", [128, 128], F32, kind="ExternalInput")

    z_d = nc.dram_tensor("z", [nb * SEG_BLK, dout], F32)
    out_d = nc.dram_tensor("out", [t2 * SEG_BLK, dout], F32, kind="ExternalOutput")

    with TileContext(nc) as tc:
        with (
            tc.tile_pool(name="const", bufs=1) as cpool,
            tc.tile_pool(name="gath", bufs=3) as gpool,
            tc.tile_pool(name="work", bufs=3) as wpool,
            tc.tile_pool(name="psA", bufs=2, space="PSUM") as psA,
            tc.tile_pool(name="psB", bufs=2, space="PSUM") as psB,
            tc.tile_pool(name="psC", bufs=2, space="PSUM") as psC,
            tc.tile_pool(name="psD", bufs=2, space="PSUM") as psD,
        ):
            gidx_sb = cpool.tile([128, nb4cols], I16)
            segs_sb = cpool.tile([128, nb * 4 * tg], F32)
            selfidx_sb = cpool.tile([128, nb], I32)
            oidx_sb = cpool.tile([128, ngath * (OUT_GATHER // 16)], I16)
            w1t_sb = cpool.tile([din, dout], F32)
            w2t_sb = cpool.tile([din, dout], F32)
            bias_sb = cpool.tile([dout, 1], F32)
            iota_sb = cpool.tile([128, SEG_BLK], F32)
            eye_sb = cpool.tile([128, 128], F32)
            for sb_t, d_t in [(gidx_sb, gidx_d), (segs_sb, segs_d),
                              (selfidx_sb, selfidx_d), (oidx_sb, oidx_d),
                              (w1t_sb, w1t_d), (w2t_sb, w2t_d), (bias_sb, bias_d),
                              (iota_sb, iota_d), (eye_sb, eye_d)]:
                nc.sync.dma_start(out=sb_t[:], in_=d_t[:])

            for b in range(nb):
                xg = []
                for c in range(4):
                    g = gpool.tile([128, tg * din], F32, tag=f"g{c}")
                    slab = (b * 4 + c) * (gsz // 16)
                    nc.gpsimd.dma_gather(
                        out_ap=g[:].rearrange("p (t d) -> p t d", d=din),
                        in_ap=x_d[c * CHUNK : min((c + 1) * CHUNK, nsrc), :],
                        idxs_ap=gidx_sb[:, slab : slab + gsz // 16],
                        num_idxs=gsz, num_idxs_reg=gsz, elem_size=din,
                        queue_num=c,
                    )
                    xg.append(g)
                xself = gpool.tile([128, din], F32, tag="self")
                nc.gpsimd.indirect_dma_start(
                    out=xself[:], out_offset=None, in_=x_d[:],
                    in_offset=bass.IndirectOffsetOnAxis(
                        ap=selfidx_sb[:, b : b + 1], axis=0),
                )

                ngT = psA.tile([din, SEG_BLK], F32, space="PSUM")
                n_mm = 4 * tg
                i_mm = 0
                for c in range(4):
                    for t in range(tg):
                        sel = wpool.tile([128, SEG_BLK], F32, tag="sel")
                        col = (b * 4 + c) * tg + t
                        nc.vector.tensor_scalar(
                            out=sel[:], in0=iota_sb[:],
                            scalar1=segs_sb[:, col : col + 1], scalar2=None,
                            op0=mybir.AluOpType.is_equal,
                        )
                        nc.tensor.matmul(
                            out=ngT[:], lhsT=xg[c][:, t * din : (t + 1) * din],
                            rhs=sel[:], start=(i_mm == 0), stop=(i_mm == n_mm - 1),
                        )
                        i_mm += 1
                selfT = psB.tile([din, 128], F32, space="PSUM")
                nc.tensor.matmul(out=selfT[:], lhsT=xself[:], rhs=eye_sb[:],
                                 start=True, stop=True)

                ngT_sb = wpool.tile([din, SEG_BLK], F32, tag="ngT")
                nc.scalar.copy(out=ngT_sb[:], in_=ngT[:])
                selfT_sb = wpool.tile([din, 128], F32, tag="selfT")
                nc.vector.tensor_copy(out=selfT_sb[:], in_=selfT[:])

                zT = psC.tile([dout, SEG_BLK], F32, space="PSUM")
                nc.tensor.matmul(out=zT[:], lhsT=w2t_sb[:], rhs=ngT_sb[:],
                                 start=True, stop=False)
                nc.tensor.matmul(out=zT[:], lhsT=w1t_sb[:], rhs=selfT_sb[:],
                                 start=False, stop=True)
                zT_sb = wpool.tile([dout, SEG_BLK], F32, tag="zT")
                nc.vector.tensor_scalar(out=zT_sb[:], in0=zT[:],
                                        scalar1=bias_sb[:], scalar2=None,
                                        op0=mybir.AluOpType.add)
                z_ps = psD.tile([SEG_BLK, dout], F32, space="PSUM")
                nc.tensor.matmul(out=z_ps[:], lhsT=zT_sb[:], rhs=eye_sb[:],
                                 start=True, stop=True)
                z_sb = wpool.tile([SEG_BLK, dout], F32, tag="z")
                nc.scalar.copy(out=z_sb[:], in_=z_ps[:])
                nc.sync.dma_start(out=z_d[b * SEG_BLK : (b + 1) * SEG_BLK, :],
                                  in_=z_sb[:])

            tc.strict_bb_all_engine_barrier()

            out_view = out_d[:].rearrange("(p t) d -> p (t d)", p=128)
            tpg = OUT_GATHER // 128          # tiles per phase-2 gather
            for g in range(ngath):
                zg = gpool.tile([128, tpg * dout], F32, tag="og")
                nc.gpsimd.dma_gather(
                    out_ap=zg[:].rearrange("p (t d) -> p t d", d=dout),
                    in_ap=z_d[:],
                    idxs_ap=oidx_sb[:, g * (OUT_GATHER // 16) : (g + 1) * (OUT_GATHER // 16)],
                    num_idxs=OUT_GATHER, num_idxs_reg=OUT_GATHER, elem_size=dout,
                    queue_num=g % 4,
                )
                nc.sync.dma_start(
                    out=out_view[:, g * tpg * dout : (g + 1) * tpg * dout],
                    in_=zg[:],
                )
    nc.finalize()
    return nc


def kernel(x, W, b, edge_src, edge_dst, self_ids, owned_ids):
    x = np.asarray(x); W = np.asarray(W); b = np.asarray(b)
    edge_src = np.asarray(edge_src); edge_dst = np.asarray(edge_dst)
    self_ids = np.asarray(self_ids); owned_ids = np.asarray(owned_ids)

    P, nsrc, din = x.shape
    ndst = int(self_ids.shape[1]) if False else int(edge_dst.max()) + 1
    # NDST must cover all dst ids referenced anywhere
    ndst = max(int(edge_dst.max()), int(owned_ids.max())) + 1
    nown = owned_ids.shape[1]
    dout = W.shape[0]

    preps = []
    for c in range(NCORES):
        p, h = c // 2, c % 2
        preps.append(_prep_core(edge_src[p], edge_dst[p], self_ids[p],
                                owned_ids[p], h, ndst))

    nb = max((pr["nu"] + SEG_BLK - 1) // SEG_BLK for pr in preps)
    nb4 = nb * 4
    gsz = 128
    for pr in preps:
        cnt = np.bincount(pr["key"], minlength=nb4)
        gsz = max(gsz, int(cnt.max()))
    gsz = ((gsz + 127) // 128) * 128
    nout_max = max(len(pr["rows"]) for pr in preps)
    t2 = ((nout_max + OUT_GATHER - 1) // OUT_GATHER) * (OUT_GATHER // SEG_BLK)

    w1t = np.ascontiguousarray(W[:, :din].T)
    w2t = np.ascontiguousarray(W[:, din:].T)
    bias = np.ascontiguousarray(b[:, None])
    iota = np.broadcast_to(np.arange(SEG_BLK, dtype=np.float32), (128, SEG_BLK))
    eye = np.eye(128, dtype=np.float32)

    in_maps = []
    for c in range(NCORES):
        st = _build_streams(preps[c], nb, gsz, t2)
        in_maps.append(dict(
            x=np.ascontiguousarray(x[c // 2]),
            gidx=st["gidx"], segs=st["segs"], selfidx=st["selfidx"],
            oidx=st["oidx"], w1t=w1t, w2t=w2t, bias=bias,
            iota=np.ascontiguousarray(iota), eye=eye,
        ))

    nc = _build_program(nsrc, din, dout, nb, gsz, t2)

    if os.environ.get("BASS_KERNEL_SIM"):
        from concourse.bass_interp import MultiCoreSim
        sim = MultiCoreSim(nc, NCORES)
        for c in range(NCORES):
            for k, v in in_maps[c].items():
                sim.cores[c].tensor(k)[:] = v
        sim.simulate()
        results = [{"out": sim.cores[c].tensor("out").copy()}
                   for c in range(NCORES)]
    else:
        from concourse.bass_utils import run_bass_kernel_spmd
        trace = bool(os.environ.get("BASS_KERNEL_TRACE"))
        if trace:
            import sys, types
            if "antenv.axon_hooks" not in sys.modules:
                mod = types.ModuleType("antenv.axon_hooks")
                mod._hook = None
                mod.set_axon_ntff_profile_hook = lambda h: setattr(mod, "_hook", h)
                mod.get_axon_ntff_profile_hook = lambda: mod._hook
                sys.modules["antenv.axon_hooks"] = mod
                import antenv
                antenv.axon_hooks = mod
                from trn_agent_boot.trn_boot import _ntff_profile_via_ctypes
                mod.set_axon_ntff_profile_hook(
                    _ntff_profile_via_ctypes("/opt/axon/libaxon_pjrt.so"))
        res = run_bass_kernel_spmd(nc, in_maps, list(range(NCORES)),
                                   trace=trace, trace_cores=[0] if trace else None)
        results = res.results
        global LAST_EXEC_NS
        LAST_EXEC_NS = res.exec_time_ns

    out = np.empty((P, nown, dout), np.float32)
    for c in range(NCORES):
        p = c // 2
        pr = preps[c]
        n = len(pr["rows"])
        j = np.arange(n)
        g = j // OUT_GATHER
        r = j % OUT_GATHER
        tl = r // 128
        pp = r % 128
        dramrow = pp * t2 + g * (OUT_GATHER // 128) + tl
        out[p, pr["rows"]] = results[c]["out"][dramrow]
    return out


# revision 5
# speedup vs baseline: 1.0407x; 1.0407x over previous
"""DistSageConv forward on 8 Trainium2 NeuronCores (Bass/Tile).

Math per graph partition p (of 4):
    ng  = segment_sum(x[edge_src], edge_dst, NDST)          # neighbor agg
    h   = concat([x[self_ids], ng], axis=1)
    out = h[owned_ids] @ W.T + b
      == x[self_ids[owned_ids]] @ W1.T + ng[owned_ids] @ W2.T + b
         (W1 = W[:, :DIN], W2 = W[:, DIN:])

Only dst nodes that appear in owned_ids matter, so edges whose dst is not
owned are dropped on the host (~60% of them). Each partition is split
across 2 cores by interleaving its unique owned dst ids ("segments").

Per core the device program is:
  phase 1, per block of 128 segments:
    - dma_gather the block's edge source rows from x (4 SWDGE queues,
      edges grouped by src chunk of 25k rows to fit int16 indices)
    - build a one-hot selection matrix SelT[e, s] = (seg_local[e] == s)
      with a vector is_equal against a host-supplied iota tile
    - PE matmul accumulate ngT[din, seg] += xs_tile.T @ SelT  (PSUM)
    - indirect-DMA gather the block's 128 self rows (partition = seg)
    - selfT = PE transpose(xself);  zT = W2T.T @ ngT + W1T.T @ selfT
    - z = PE transpose(zT + b)  ->  DMA to z_dram[block*128 : ...]
  phase 2:
    - dma_gather z rows for this core's output rows, write them to the
      output buffer in a host-chosen order (host unpermutes at the end).
"""
import os
import numpy as np

import concourse.bass as bass
import concourse.bacc as bacc
import concourse.mybir as mybir
from concourse.tile import TileContext

F32 = mybir.dt.float32
I32 = mybir.dt.int32
I16 = mybir.dt.int16

# Tile's sem assignment round-robins SWDGE DMA insts across DMASW lanes
# with no regard for queue_num, but each DMA semaphore may only be updated
# from one SWDGE queue. Pin lane = queue_num so multi-queue gathers are
# legal. (Insts without queue_num, e.g. indirect_dma_start on qPoolDynamic,
# run on SWDGE queue 0 and get lane 0.)
import concourse.tile_sem_assignment as _tsa

if not getattr(_tsa, "_queue_lane_patch", False):
    _orig_assign_tick = _tsa.TileClockTick._assign_tick

    def _assign_tick_queue_aware(self, inst):
        if (
            isinstance(inst, _tsa.DMAInst)
            and inst.engine == mybir.EngineType.Pool
        ):
            self.next_sw_dma_idx = getattr(inst, "queue_num", 0) or 0
        return _orig_assign_tick(self, inst)

    _tsa.TileClockTick._assign_tick = _assign_tick_queue_aware
    _tsa._queue_lane_patch = True

NCORES = 8
LAST_EXEC_NS = None
SEG_BLK = 128          # segments per block (= psum tile free dim)
CHUNK = 25000          # src rows per dma_gather chunk (int16 idx < 32768)
OUT_GATHER = 1024      # rows per phase-2 dma_gather


def _wrap16(flat):
    """dma_gather index layout: idx i -> [i % 16, i // 16], replicated to
    all 8 groups of 16 partitions. flat length must be a multiple of 16."""
    n = len(flat)
    w = flat.reshape(n // 16, 16).T          # [16, n/16]
    return np.tile(w, (8, 1))                # [128, n/16]


def _prep_core(es, ed, sid, oid, half, ndst):
    """Host-side index prep for one core (partition p, half h)."""
    uniq = np.unique(oid)
    U = uniq[half::2]                         # this core's segments, sorted
    nu = len(U)
    seg_of_dst = np.full(ndst, -1, np.int32)
    seg_of_dst[U] = np.arange(nu, dtype=np.int32)

    seg_all = seg_of_dst[ed]
    keep = seg_all >= 0
    es_k = es[keep].astype(np.int64)
    seg_k = seg_all[keep].astype(np.int64)
    blk = seg_k // SEG_BLK
    loc = (seg_k % SEG_BLK).astype(np.float32)
    ch = es_k // CHUNK

    order = np.lexsort((ch, blk))
    es_o = (es_k - ch * CHUNK)[order]
    loc_o = loc[order]
    key_o = (blk * 4 + ch)[order]             # sorted ascending

    self_src = sid[U]                          # [nu]

    seg_out = seg_of_dst[oid]                  # [NOWN], -1 if other core's
    mine = seg_out >= 0
    rows = np.nonzero(mine)[0]                 # original output positions
    oseg = seg_out[mine].astype(np.int64)      # global seg id per out row

    return dict(nu=nu, es=es_o, loc=loc_o, key=key_o,
                self_src=self_src, rows=rows, oseg=oseg)


def _build_streams(prep, nb, gsz, t2):
    """Per-core device input arrays for the fixed (nb, gsz, t2) structure."""
    nb4 = nb * 4
    # grouped edge arrays [nb4, gsz]
    G = np.zeros((nb4, gsz), np.int16)
    S = np.full((nb4, gsz), -1.0, np.float32)
    key, es, loc = prep["key"], prep["es"], prep["loc"]
    starts = np.searchsorted(key, np.arange(nb4 + 1))
    ofs = np.arange(len(key)) - starts[key]
    G[key, ofs] = es.astype(np.int16)
    S[key, ofs] = loc

    # gather idx stream: per (b,ch) slab of gsz/16 cols, wrapped
    gidx = np.concatenate([_wrap16(G[i]) for i in range(nb4)], axis=1)
    # seg stream: [128, nb4 * gsz/128], col = slab*(gsz/128) + tile
    segs = np.ascontiguousarray(
        S.reshape(nb4, gsz // SEG_BLK, SEG_BLK).transpose(2, 0, 1).reshape(128, -1)
    )

    # self stream [128, nb] int32 (partition = seg local, col = block)
    selfidx = np.zeros((nb * SEG_BLK,), np.int32)
    selfidx[: prep["nu"]] = prep["self_src"]
    selfidx = np.ascontiguousarray(selfidx.reshape(nb, SEG_BLK).T)

    # phase-2 gather idx [128, t2*128/16] int16 (z rows = global seg id)
    oflat = np.zeros((t2 * SEG_BLK,), np.int64)
    oflat[: len(prep["oseg"])] = prep["oseg"]
    ngath = (t2 * SEG_BLK) // OUT_GATHER
    oidx = np.concatenate(
        [_wrap16(oflat[g * OUT_GATHER : (g + 1) * OUT_GATHER].astype(np.int16))
         for g in range(ngath)],
        axis=1,
    )
    return dict(gidx=np.ascontiguousarray(gidx), segs=segs,
                selfidx=selfidx, oidx=np.ascontiguousarray(oidx))


def _build_program(nsrc, din, dout, nb, gsz, t2):
    nc = bacc.Bacc(num_swdge_queues=4)
    tg = gsz // 128                      # tiles per (block, chunk) gather
    nb4cols = nb * 4 * (gsz // 16)
    x_d = nc.dram_tensor("x", [nsrc, din], F32, kind="ExternalInput")
    gidx_d = nc.dram_tensor("gidx", [128, nb4cols], I16, kind="ExternalInput")
    segs_d = nc.dram_tensor("segs", [128, nb * 4 * tg], F32, kind="ExternalInput")
    selfidx_d = nc.dram_tensor("selfidx", [128, nb], I32, kind="ExternalInput")
    ngath = (t2 * SEG_BLK) // OUT_GATHER
    oidx_d = nc.dram_tensor("oidx", [128, ngath * (OUT_GATHER // 16)], I16,
                            kind="ExternalInput")
    w1t_d = nc.dram_tensor("w1t", [din, dout], F32, kind="ExternalInput")
    w2t_d = nc.dram_tensor("w2t", [din, dout], F32, kind="ExternalInput")
    bias_d = nc.dram_tensor("bias", [dout, 1], F32, kind="ExternalInput")
    iota_d = nc.dram_tensor("iota", [128, SEG_BLK], F32, kind="ExternalInput")
    eye_d = nc.dram_tensor("eye", [128, 128], F32, kind="ExternalInput")

    z_d = nc.dram_tensor("z", [nb * SEG_BLK, dout], F32)
    out_d = nc.dram_tensor("out", [t2 * SEG_BLK, dout], F32, kind="ExternalOutput")

    with TileContext(nc) as tc:
        with (
            tc.tile_pool(name="const", bufs=1) as cpool,
            tc.tile_pool(name="gath", bufs=3) as gpool,
            tc.tile_pool(name="work", bufs=3) as wpool,
            tc.tile_pool(name="psA", bufs=2, space="PSUM") as psA,
            tc.tile_pool(name="psB", bufs=2, space="PSUM") as psB,
            tc.tile_pool(name="psC", bufs=2, space="PSUM") as psC,
            tc.tile_pool(name="psD", bufs=2, space="PSUM") as psD,
        ):
            gidx_sb = cpool.tile([128, nb4cols], I16)
            segs_sb = cpool.tile([128, nb * 4 * tg], F32)
            selfidx_sb = cpool.tile([128, nb], I32)
            oidx_sb = cpool.tile([128, ngath * (OUT_GATHER // 16)], I16)
            w1t_sb = cpool.tile([din, dout], F32)
            w2t_sb = cpool.tile([din, dout], F32)
            bias_sb = cpool.tile([dout, 1], F32)
            iota_sb = cpool.tile([128, SEG_BLK], F32)
            eye_sb = cpool.tile([128, 128], F32)
            for sb_t, d_t in [(gidx_sb, gidx_d), (segs_sb, segs_d),
                              (selfidx_sb, selfidx_d), (oidx_sb, oidx_d),
                              (w1t_sb, w1t_d), (w2t_sb, w2t_d), (bias_sb, bias_d),
                              (iota_sb, iota_d), (eye_sb, eye_d)]:
                nc.sync.dma_start(out=sb_t[:], in_=d_t[:])

            for b in range(nb):
                xg = []
                for c in range(4):
                    g = gpool.tile([128, tg * din], F32, tag=f"g{c}")
                    slab = (b * 4 + c) * (gsz // 16)
                    nc.gpsimd.dma_gather(
                        out_ap=g[:].rearrange("p (t d) -> p t d", d=din),
                        in_ap=x_d[c * CHUNK : min((c + 1) * CHUNK, nsrc), :],
                        idxs_ap=gidx_sb[:, slab : slab + gsz // 16],
                        num_idxs=gsz, num_idxs_reg=gsz, elem_size=din,
                        queue_num=c,
                    )
                    xg.append(g)
                xself = gpool.tile([128, din], F32, tag="self")
                nc.gpsimd.indirect_dma_start(
                    out=xself[:], out_offset=None, in_=x_d[:],
                    in_offset=bass.IndirectOffsetOnAxis(
                        ap=selfidx_sb[:, b : b + 1], axis=0),
                )

                ngT = psA.tile([din, SEG_BLK], F32, space="PSUM")
                n_mm = 4 * tg
                i_mm = 0
                for c in range(4):
                    for t in range(tg):
                        sel = wpool.tile([128, SEG_BLK], F32, tag="sel")
                        col = (b * 4 + c) * tg + t
                        nc.vector.tensor_scalar(
                            out=sel[:], in0=iota_sb[:],
                            scalar1=segs_sb[:, col : col + 1], scalar2=None,
                            op0=mybir.AluOpType.is_equal,
                        )
                        nc.tensor.matmul(
                            out=ngT[:], lhsT=xg[c][:, t * din : (t + 1) * din],
                            rhs=sel[:], start=(i_mm == 0), stop=(i_mm == n_mm - 1),
                        )
                        i_mm += 1
                selfT = psB.tile([din, 128], F32, space="PSUM")
                nc.tensor.matmul(out=selfT[:], lhsT=xself[:], rhs=eye_sb[:],
                                 start=True, stop=True)

                ngT_sb = wpool.tile([din, SEG_BLK], F32, tag="ngT")
                nc.scalar.copy(out=ngT_sb[:], in_=ngT[:])
                selfT_sb = wpool.tile([din, 128], F32, tag="selfT")
                nc.vector.tensor_copy(out=selfT_sb[:], in_=selfT[:])

                zT = psC.tile([dout, SEG_BLK], F32, space="PSUM")
                nc.tensor.matmul(out=zT[:], lhsT=w2t_sb[:], rhs=ngT_sb[:],
                                 start=True, stop=False)
                nc.tensor.matmul(out=zT[:], lhsT=w1t_sb[:], rhs=selfT_sb[:],
                                 start=False, stop=True)
                zT_sb = wpool.tile([dout, SEG_BLK], F32, tag="zT")
                nc.vector.tensor_scalar(out=zT_sb[:], in0=zT[:],
                                        scalar1=bias_sb[:], scalar2=None,
                                        op0=mybir.AluOpType.add)
                z_ps = psD.tile([SEG_BLK, dout], F32, space="PSUM")
                nc.tensor.matmul(out=z_ps[:], lhsT=zT_sb[:], rhs=eye_sb[:],
                                 start=True, stop=True)
                z_sb = wpool.tile([SEG_BLK, dout], F32, tag="z")
                nc.scalar.copy(out=z_sb[:], in_=z_ps[:])
                nc.sync.dma_start(out=z_d[b * SEG_BLK : (b + 1) * SEG_BLK, :],
                                  in_=z_sb[:])

            tc.strict_bb_all_engine_barrier()

            out_view = out_d[:].rearrange("(p t) d -> p (t d)", p=128)
            tpg = OUT_GATHER // 128          # tiles per phase-2 gather
            for g in range(ngath):
                zg = gpool.tile([128, tpg * dout], F32, tag="og")
                nc.gpsimd.dma_gather(
                    out_ap=zg[:].rearrange("p (t d) -> p t d", d=dout),
                    in_ap=z_d[:],
                    idxs_ap=oidx_sb[:, g * (OUT_GATHER // 16) : (g + 1) * (OUT_GATHER // 16)],
                    num_idxs=OUT_GATHER, num_idxs_reg=OUT_GATHER, elem_size=dout,
                    queue_num=g % 4,
                )
                nc.sync.dma_start(
                    out=out_view[:, g * tpg * dout : (g + 1) * tpg * dout],
                    in_=zg[:],
                )
    nc.finalize()
    return nc


def kernel(x, W, b, edge_src, edge_dst, self_ids, owned_ids):
    x = np.asarray(x); W = np.asarray(W); b = np.asarray(b)
    edge_src = np.asarray(edge_src); edge_dst = np.asarray(edge_dst)
    self_ids = np.asarray(self_ids); owned_ids = np.asarray(owned_ids)

    P, nsrc, din = x.shape
    ndst = int(self_ids.shape[1]) if False else int(edge_dst.max()) + 1
    # NDST must cover all dst ids referenced anywhere
    ndst = max(int(edge_dst.max()), int(owned_ids.max())) + 1
    nown = owned_ids.shape[1]
    dout = W.shape[0]

    preps = []
    for c in range(NCORES):
        p, h = c // 2, c % 2
        preps.append(_prep_core(edge_src[p], edge_dst[p], self_ids[p],
                                owned_ids[p], h, ndst))

    nb = max((pr["nu"] + SEG_BLK - 1) // SEG_BLK for pr in preps)
    nb4 = nb * 4
    gsz = 128
    for pr in preps:
        cnt = np.bincount(pr["key"], minlength=nb4)
        gsz = max(gsz, int(cnt.max()))
    gsz = ((gsz + 127) // 128) * 128
    nout_max = max(len(pr["rows"]) for pr in preps)
    t2 = ((nout_max + OUT_GATHER - 1) // OUT_GATHER) * (OUT_GATHER // SEG_BLK)

    w1t = np.ascontiguousarray(W[:, :din].T)
    w2t = np.ascontiguousarray(W[:, din:].T)
    bias = np.ascontiguousarray(b[:, None])
    iota = np.broadcast_to(np.arange(SEG_BLK, dtype=np.float32), (128, SEG_BLK))
    eye = np.eye(128, dtype=np.float32)

    in_maps = []
    for c in range(NCORES):
        st = _build_streams(preps[c], nb, gsz, t2)
        in_maps.append(dict(
            x=np.ascontiguousarray(x[c // 2]),
            gidx=st["gidx"], segs=st["segs"], selfidx=st["selfidx"],
            oidx=st["oidx"], w1t=w1t, w2t=w2t, bias=bias,
            iota=np.ascontiguousarray(iota), eye=eye,
        ))

    nc = _build_program(nsrc, din, dout, nb, gsz, t2)

    if os.environ.get("BASS_KERNEL_SIM"):
        from concourse.bass_interp import MultiCoreSim
        sim = MultiCoreSim(nc, NCORES)
        for c in range(NCORES):
            for k, v in in_maps[c].items():
                sim.cores[c].tensor(k)[:] = v
        sim.simulate()
        results = [{"out": sim.cores[c].tensor("out").copy()}
                   for c in range(NCORES)]
    else:
        from concourse.bass_utils import run_bass_kernel_spmd
        trace = bool(os.environ.get("BASS_KERNEL_TRACE"))
        if trace:
            import sys, types
            if "antenv.axon_hooks" not in sys.modules:
                mod = types.ModuleType("antenv.axon_hooks")
                mod._hook = None
                mod.set_axon_ntff_profile_hook = lambda h: setattr(mod, "_hook", h)
                mod.get_axon_ntff_profile_hook = lambda: mod._hook
                sys.modules["antenv.axon_hooks"] = mod
                import antenv
                antenv.axon_hooks = mod
                from trn_agent_boot.trn_boot import _ntff_profile_via_ctypes
                mod.set_axon_ntff_profile_hook(
                    _ntff_profile_via_ctypes("/opt/axon/libaxon_pjrt.so"))
        res = run_bass_kernel_spmd(nc, in_maps, list(range(NCORES)),
                                   trace=trace, trace_cores=[0] if trace else None,
                                   tmpdir=os.environ.get("BASS_KERNEL_TRACE_DIR"))
        results = res.results
        global LAST_EXEC_NS
        LAST_EXEC_NS = res.exec_time_ns

    out = np.empty((P, nown, dout), np.float32)
    for c in range(NCORES):
        p = c // 2
        pr = preps[c]
        n = len(pr["rows"])
        j = np.arange(n)
        g = j // OUT_GATHER
        r = j % OUT_GATHER
        tl = r // 128
        pp = r % 128
        dramrow = pp * t2 + g * (OUT_GATHER // 128) + tl
        out[p, pr["rows"]] = results[c]["out"][dramrow]
    return out


# revision 10
# speedup vs baseline: 1.4092x; 1.3542x over previous
"""DistSageConv forward on 8 Trainium2 NeuronCores (Bass/Tile).

Math per graph partition p (of 4):
    ng  = segment_sum(x[edge_src], edge_dst, NDST)          # neighbor agg
    out = x[self_ids[owned_ids]] @ W1.T + ng[owned_ids] @ W2.T + b
          (W1 = W[:, :DIN], W2 = W[:, DIN:])

Only dst nodes appearing in owned_ids matter, so edges to non-owned dst are
dropped on the host (~60%). Each partition is split across 2 cores by
interleaving its unique owned dst ids ("segments"); segments are processed
in blocks of 128.

Per core, per block (device):
  - dma_gather the block's edge source rows from x (bf16, one gather per
    src chunk on its own SWDGE queue — 4 queues parallelize Q7 descriptor
    generation, which is the throughput limiter for row gathers)
  - SelT[e, s] = (seg_local[e] == s) via vector is_equal against an iota
  - PE: ngT[din, seg] += xs_tile.T @ SelT   (PSUM, fp32 accumulate)
  - indirect-DMA the block's 128 self rows (partition = seg local)
  - selfT = PE transpose(xself);  zT = W2T.T @ ngT + W1T.T @ selfT
  - z = PE transpose(zT + b)  ->  DMA to z_dram (fp32)
Phase 2: dma_gather z rows for this core's output rows into a host-chosen
order; host scatters them back into the [P, NOWN, DOUT] result.
"""
import os
import numpy as np
import ml_dtypes

import concourse.bass as bass
import concourse.bacc as bacc
import concourse.mybir as mybir
from concourse.tile import TileContext

F32 = mybir.dt.float32
BF16 = mybir.dt.float16
I32 = mybir.dt.int32
I16 = mybir.dt.int16
BF16_NP = np.float16

NCORES = 8
LAST_EXEC_NS = None
SEG_BLK = 128
# src chunk boundaries as fractions of NSRC (chunk sizes must stay <32768
# for int16 gather indices; chunk 0 is smaller because queue 0 also carries
# the per-block self-row gathers)
CHUNK_FRACS = (0.0, 0.18, 0.4533, 0.7266, 1.0)
OUT_GATHER = 1024
RING = 4

# Tile's sem assignment round-robins SWDGE DMA insts across DMASW lanes
# with no regard for queue_num, but each DMA semaphore may only be updated
# from one SWDGE queue. Pin lane = queue_num so multi-queue gathers are
# legal. (Insts without queue_num, e.g. indirect_dma_start on qPoolDynamic,
# run on SWDGE queue 0 and get lane 0.)
import concourse.tile_sem_assignment as _tsa

if not getattr(_tsa, "_queue_lane_patch", False):
    _orig_assign_tick = _tsa.TileClockTick._assign_tick

    def _assign_tick_queue_aware(self, inst):
        if (
            isinstance(inst, _tsa.DMAInst)
            and inst.engine == mybir.EngineType.Pool
        ):
            self.next_sw_dma_idx = getattr(inst, "queue_num", 0) or 0
        return _orig_assign_tick(self, inst)

    _tsa.TileClockTick._assign_tick = _assign_tick_queue_aware
    _tsa._queue_lane_patch = True


def _wrap16(flat):
    """dma_gather index layout: idx i -> [i % 16, i // 16], replicated to
    all 8 groups of 16 partitions. len(flat) must be a multiple of 16."""
    n = len(flat)
    w = flat.reshape(n // 16, 16).T
    return np.tile(w, (8, 1))


def _chunk_cuts(nsrc):
    cuts = [int(round(f * nsrc)) for f in CHUNK_FRACS]
    cuts[0], cuts[-1] = 0, nsrc
    for a, b in zip(cuts, cuts[1:]):
        assert 0 < b - a < 32768
    return np.array(cuts, np.int64)


def _prep_core(es, ed, sid, oid, half, ndst, cuts):
    """Host-side index prep for one core (partition p, half h)."""
    uniq = np.unique(oid)
    U = uniq[half::2]
    nu = len(U)
    seg_of_dst = np.full(ndst, -1, np.int32)
    seg_of_dst[U] = np.arange(nu, dtype=np.int32)

    seg_all = seg_of_dst[ed]
    keep = seg_all >= 0
    es_k = es[keep].astype(np.int64)
    seg_k = seg_all[keep].astype(np.int64)
    blk = seg_k // SEG_BLK
    loc = (seg_k % SEG_BLK).astype(np.float32)
    ch = np.searchsorted(cuts, es_k, side="right") - 1

    order = np.lexsort((ch, blk))
    es_o = (es_k - cuts[ch])[order]
    loc_o = loc[order]
    key_o = (blk * 4 + ch)[order]

    self_src = sid[U]
    seg_out = seg_of_dst[oid]
    mine = seg_out >= 0
    rows = np.nonzero(mine)[0]
    oseg = seg_out[mine].astype(np.int64)
    return dict(nu=nu, es=es_o, loc=loc_o, key=key_o,
                self_src=self_src, rows=rows, oseg=oseg)


def _slab_sizes(preps, nb):
    """Static per-(block, chunk) gather sizes: max edge count over cores,
    rounded up to 16 (dma_gather idx wrap granularity)."""
    nb4 = nb * 4
    gmax = np.zeros(nb4, np.int64)
    for pr in preps:
        cnt = np.bincount(pr["key"], minlength=nb4)
        gmax = np.maximum(gmax, cnt)
    nidx = ((gmax + 15) // 16) * 16
    # make sure every block has at least one tile so its ngT psum is written
    for b in range(nb):
        if nidx[b * 4 : (b + 1) * 4].sum() == 0:
            nidx[b * 4] = 16
    tiles = (nidx + 127) // 128
    return nidx.astype(int), tiles.astype(int)


def _build_streams(prep, nb, nidx, tiles, t2):
    nb4 = nb * 4
    key, es, loc = prep["key"], prep["es"], prep["loc"]
    starts = np.searchsorted(key, np.arange(nb4 + 1))
    ofs = np.arange(len(key)) - starts[key]

    gidx_parts, seg_parts = [], []
    for s in range(nb4):
        n, nt = int(nidx[s]), int(tiles[s])
        sl = slice(starts[s], starts[s + 1])
        g = np.zeros(n, np.int16)
        g[ofs[sl]] = es[sl].astype(np.int16)
        gidx_parts.append(_wrap16(g) if n else np.zeros((128, 0), np.int16))
        sv = np.full(nt * 128, -1.0, np.float32)
        sv[ofs[sl]] = loc[sl]
        seg_parts.append(sv.reshape(nt, 128).T)
    gidx = np.concatenate(gidx_parts, axis=1) if gidx_parts else np.zeros((128, 0), np.int16)
    segs = np.concatenate(seg_parts, axis=1).astype(np.float32)

    selfidx = np.zeros((nb * SEG_BLK,), np.int32)
    selfidx[: prep["nu"]] = prep["self_src"]
    selfidx = np.ascontiguousarray(selfidx.reshape(nb, SEG_BLK).T)

    oflat = np.zeros((t2 * SEG_BLK,), np.int64)
    oflat[: len(prep["oseg"])] = prep["oseg"]
    ngath = (t2 * SEG_BLK) // OUT_GATHER
    oidx = np.concatenate(
        [_wrap16(oflat[g * OUT_GATHER : (g + 1) * OUT_GATHER].astype(np.int16))
         for g in range(ngath)],
        axis=1,
    )
    return dict(gidx=np.ascontiguousarray(gidx), segs=np.ascontiguousarray(segs),
                selfidx=selfidx, oidx=np.ascontiguousarray(oidx))


def _build_program(nsrc, din, dout, nb, nidx, tiles, t2, cuts):
    nc = bacc.Bacc(num_swdge_queues=4)
    nb4 = nb * 4
    gcols = int(sum(n // 16 for n in nidx))
    scols = int(tiles.sum())
    tmax = int(tiles.max())
    ngath = (t2 * SEG_BLK) // OUT_GATHER

    x_d = nc.dram_tensor("x", [nsrc, din], BF16, kind="ExternalInput")
    gidx_d = nc.dram_tensor("gidx", [128, max(gcols, 1)], I16, kind="ExternalInput")
    segs_d = nc.dram_tensor("segs", [128, scols], F32, kind="ExternalInput")
    selfidx_d = nc.dram_tensor("selfidx", [128, nb], I32, kind="ExternalInput")
    oidx_d = nc.dram_tensor("oidx", [128, ngath * (OUT_GATHER // 16)], I16,
                            kind="ExternalInput")
    w1t_d = nc.dram_tensor("w1t", [din, dout], BF16, kind="ExternalInput")
    w2t_d = nc.dram_tensor("w2t", [din, dout], BF16, kind="ExternalInput")
    bias_d = nc.dram_tensor("bias", [dout, 1], F32, kind="ExternalInput")
    iota_d = nc.dram_tensor("iota", [128, SEG_BLK], BF16, kind="ExternalInput")
    eye16_d = nc.dram_tensor("eye16", [128, 128], BF16, kind="ExternalInput")
    eye32_d = nc.dram_tensor("eye32", [128, 128], F32, kind="ExternalInput")

    z_d = nc.dram_tensor("z", [nb * SEG_BLK, dout], F32)
    out_d = nc.dram_tensor("out", [t2 * SEG_BLK, dout], F32, kind="ExternalOutput")

    # column offsets per slab
    goff = np.concatenate([[0], np.cumsum([n // 16 for n in nidx])]).astype(int)
    soff = np.concatenate([[0], np.cumsum(tiles)]).astype(int)

    with TileContext(nc) as tc:
        with (
            tc.tile_pool(name="const", bufs=1) as cpool,
            tc.tile_pool(name="sgath", bufs=3) as sgpool,
            tc.tile_pool(name="ogath", bufs=3) as ogpool,
            tc.tile_pool(name="sel", bufs=6) as selpool,
            tc.tile_pool(name="work", bufs=3) as wpool,
            tc.tile_pool(name="psA", bufs=2, space="PSUM") as psA,
            tc.tile_pool(name="psB", bufs=2, space="PSUM") as psB,
            tc.tile_pool(name="psC", bufs=2, space="PSUM") as psC,
            tc.tile_pool(name="psD", bufs=2, space="PSUM") as psD,
        ):
            gidx_sb = cpool.tile([128, max(gcols, 1)], I16)
            segs_sb = cpool.tile([128, scols], F32)
            selfidx_sb = cpool.tile([128, nb], I32)
            oidx_sb = cpool.tile([128, ngath * (OUT_GATHER // 16)], I16)
            w1t_sb = cpool.tile([din, dout], BF16)
            w2t_sb = cpool.tile([din, dout], BF16)
            bias_sb = cpool.tile([dout, 1], F32)
            iota_sb = cpool.tile([128, SEG_BLK], BF16)
            eye16_sb = cpool.tile([128, 128], BF16)
            eye32_sb = cpool.tile([128, 128], F32)
            for sb_t, d_t in [(gidx_sb, gidx_d), (segs_sb, segs_d),
                              (selfidx_sb, selfidx_d), (oidx_sb, oidx_d),
                              (w1t_sb, w1t_d), (w2t_sb, w2t_d), (bias_sb, bias_d),
                              (iota_sb, iota_d), (eye16_sb, eye16_d),
                              (eye32_sb, eye32_d)]:
                nc.sync.dma_start(out=sb_t[:], in_=d_t[:])

            # explicit gather ring, memset once so never-gathered tail rows
            # of a partial tile are 0.0 (SelT weight 0 keeps them out of the
            # sums; memset guarantees no stale NaN bit patterns on first use)
            ring = [[cpool.tile([128, tmax * din], BF16, tag=f"ring{c}_{r}",
                                name=f"ring{c}_{r}")
                     for r in range(RING)] for c in range(4)]
            for c in range(4):
                for r in range(RING):
                    nc.gpsimd.memset(ring[c][r][:], 0.0)

            for b in range(nb):
                xg = []
                for c in range(4):
                    s = b * 4 + c
                    n, nt = int(nidx[s]), int(tiles[s])
                    g = ring[c][b % RING]
                    if n:
                        nc.gpsimd.dma_gather(
                            out_ap=g[:, : nt * din].rearrange("p (t d) -> p t d", d=din),
                            in_ap=x_d[int(cuts[c]) : int(cuts[c + 1]), :],
                            idxs_ap=gidx_sb[:, goff[s] : goff[s + 1]],
                            num_idxs=n, num_idxs_reg=n, elem_size=din,
                            queue_num=c,
                        )
                    xg.append(g)
                xself = sgpool.tile([128, din], BF16, tag="self")
                nc.gpsimd.indirect_dma_start(
                    out=xself[:], out_offset=None, in_=x_d[:],
                    in_offset=bass.IndirectOffsetOnAxis(
                        ap=selfidx_sb[:, b : b + 1], axis=0),
                )

                ngT = psA.tile([din, SEG_BLK], F32, space="PSUM")
                n_mm = int(tiles[b * 4 : (b + 1) * 4].sum())
                i_mm = 0
                for c in range(4):
                    s = b * 4 + c
                    for t in range(int(tiles[s])):
                        sel = selpool.tile([128, SEG_BLK], BF16, tag="sel")
                        col = soff[s] + t
                        nc.vector.tensor_scalar(
                            out=sel[:], in0=iota_sb[:],
                            scalar1=segs_sb[:, col : col + 1], scalar2=None,
                            op0=mybir.AluOpType.is_equal,
                        )
                        nc.tensor.matmul(
                            out=ngT[:], lhsT=xg[c][:, t * din : (t + 1) * din],
                            rhs=sel[:], start=(i_mm == 0), stop=(i_mm == n_mm - 1),
                        )
                        i_mm += 1
                selfT = psB.tile([din, 128], F32, space="PSUM")
                nc.tensor.matmul(out=selfT[:], lhsT=xself[:], rhs=eye16_sb[:],
                                 start=True, stop=True)

                ngT_sb = wpool.tile([din, SEG_BLK], BF16, tag="ngT")
                nc.scalar.copy(out=ngT_sb[:], in_=ngT[:])
                selfT_sb = wpool.tile([din, 128], BF16, tag="selfT")
                nc.vector.tensor_copy(out=selfT_sb[:], in_=selfT[:])

                zT = psC.tile([dout, SEG_BLK], F32, space="PSUM")
                nc.tensor.matmul(out=zT[:], lhsT=w2t_sb[:], rhs=ngT_sb[:],
                                 start=True, stop=False)
                nc.tensor.matmul(out=zT[:], lhsT=w1t_sb[:], rhs=selfT_sb[:],
                                 start=False, stop=True)
                zT_sb = wpool.tile([dout, SEG_BLK], F32, tag="zT")
                nc.scalar.activation(out=zT_sb[:], in_=zT[:],
                                     func=mybir.ActivationFunctionType.Identity,
                                     bias=bias_sb[:])
                z_ps = psD.tile([SEG_BLK, dout], F32, space="PSUM")
                nc.tensor.matmul(out=z_ps[:], lhsT=zT_sb[:], rhs=eye32_sb[:],
                                 start=True, stop=True)
                z_sb = wpool.tile([SEG_BLK, dout], F32, tag="z")
                nc.vector.tensor_copy(out=z_sb[:], in_=z_ps[:])
                nc.sync.dma_start(out=z_d[b * SEG_BLK : (b + 1) * SEG_BLK, :],
                                  in_=z_sb[:])

            tc.strict_bb_all_engine_barrier()

            out_view = out_d[:].rearrange("(p t) d -> p (t d)", p=128)
            tpg = OUT_GATHER // 128
            for g in range(ngath):
                zg = ogpool.tile([128, tpg * dout], F32, tag="og")
                nc.gpsimd.dma_gather(
                    out_ap=zg[:].rearrange("p (t d) -> p t d", d=dout),
                    in_ap=z_d[:],
                    idxs_ap=oidx_sb[:, g * (OUT_GATHER // 16) : (g + 1) * (OUT_GATHER // 16)],
                    num_idxs=OUT_GATHER, num_idxs_reg=OUT_GATHER, elem_size=dout,
                    queue_num=1 + (g % 3),
                )
                nc.sync.dma_start(
                    out=out_view[:, g * tpg * dout : (g + 1) * tpg * dout],
                    in_=zg[:],
                )
    nc.finalize()
    return nc


def kernel(x, W, b, edge_src, edge_dst, self_ids, owned_ids):
    x = np.asarray(x); W = np.asarray(W); b = np.asarray(b)
    edge_src = np.asarray(edge_src); edge_dst = np.asarray(edge_dst)
    self_ids = np.asarray(self_ids); owned_ids = np.asarray(owned_ids)

    P, nsrc, din = x.shape
    ndst = max(int(edge_dst.max()), int(owned_ids.max())) + 1
    nown = owned_ids.shape[1]
    dout = W.shape[0]
    cuts = _chunk_cuts(nsrc)

    preps = []
    for c in range(NCORES):
        p, h = c // 2, c % 2
        preps.append(_prep_core(edge_src[p], edge_dst[p], self_ids[p],
                                owned_ids[p], h, ndst, cuts))

    nb = max((pr["nu"] + SEG_BLK - 1) // SEG_BLK for pr in preps)
    nidx, tiles = _slab_sizes(preps, nb)
    nout_max = max(len(pr["rows"]) for pr in preps)
    t2 = ((nout_max + OUT_GATHER - 1) // OUT_GATHER) * (OUT_GATHER // SEG_BLK)

    w1t = np.ascontiguousarray(W[:, :din].T).astype(BF16_NP)
    w2t = np.ascontiguousarray(W[:, din:].T).astype(BF16_NP)
    bias = np.ascontiguousarray(b[:, None]).astype(np.float32)
    iota = np.broadcast_to(np.arange(SEG_BLK, dtype=np.float32),
                           (128, SEG_BLK)).astype(BF16_NP)
    eye16 = np.eye(128, dtype=np.float32).astype(BF16_NP)
    eye32 = np.eye(128, dtype=np.float32)

    in_maps = []
    for c in range(NCORES):
        st = _build_streams(preps[c], nb, nidx, tiles, t2)
        in_maps.append(dict(
            x=np.ascontiguousarray(x[c // 2]).astype(BF16_NP),
            gidx=st["gidx"] if st["gidx"].shape[1] else np.zeros((128, 1), np.int16),
            segs=st["segs"], selfidx=st["selfidx"],
            oidx=st["oidx"], w1t=w1t, w2t=w2t, bias=bias,
            iota=np.ascontiguousarray(iota), eye16=eye16, eye32=eye32,
        ))

    nc = _build_program(nsrc, din, dout, nb, nidx, tiles, t2, cuts)

    if os.environ.get("BASS_KERNEL_SIM"):
        from concourse.bass_interp import MultiCoreSim
        sim = MultiCoreSim(nc, NCORES)
        for c in range(NCORES):
            for k, v in in_maps[c].items():
                sim.cores[c].tensor(k)[:] = v
        sim.simulate()
        results = [{"out": sim.cores[c].tensor("out").copy()}
                   for c in range(NCORES)]
    else:
        from concourse.bass_utils import run_bass_kernel_spmd
        trace = bool(os.environ.get("BASS_KERNEL_TRACE"))
        if trace:
            import sys, types
            if "antenv.axon_hooks" not in sys.modules:
                mod = types.ModuleType("antenv.axon_hooks")
                mod._hook = None
                mod.set_axon_ntff_profile_hook = lambda h: setattr(mod, "_hook", h)
                mod.get_axon_ntff_profile_hook = lambda: mod._hook
                sys.modules["antenv.axon_hooks"] = mod
                import antenv
                antenv.axon_hooks = mod
                from trn_agent_boot.trn_boot import _ntff_profile_via_ctypes
                mod.set_axon_ntff_profile_hook(
                    _ntff_profile_via_ctypes("/opt/axon/libaxon_pjrt.so"))
        res = run_bass_kernel_spmd(nc, in_maps, list(range(NCORES)),
                                   trace=trace, trace_cores=[0] if trace else None,
                                   tmpdir=os.environ.get("BASS_KERNEL_TRACE_DIR"))
        results = res.results
        global LAST_EXEC_NS
        LAST_EXEC_NS = res.exec_time_ns

    out = np.empty((P, nown, dout), np.float32)
    for c in range(NCORES):
        p = c // 2
        pr = preps[c]
        n = len(pr["rows"])
        j = np.arange(n)
        g = j // OUT_GATHER
        r = j % OUT_GATHER
        tl = r // 128
        pp = r % 128
        dramrow = pp * t2 + g * (OUT_GATHER // 128) + tl
        out[p, pr["rows"]] = results[c]["out"][dramrow]
    return out
